# revision 10
# baseline (speedup 1.0000x reference)
import math
from contextlib import ExitStack

import numpy as np

import concourse.bass as bass
import concourse.tile as tile
from concourse import bacc, mybir
from concourse.masks import make_identity

F32 = mybir.dt.float32
F32R = mybir.dt.float32r
BF16 = mybir.dt.bfloat16
F8 = mybir.dt.float8e4
PM = mybir.MatmulPerfMode
AL = mybir.AluOpType
AF = mybir.ActivationFunctionType

C = 192          # channels
HEADS = 4
CH = C // HEADS  # 48
W = 128          # image width
SR = 16          # rows per stripe
PW = W + 2       # padded width (ypA)
PWQ = 144        # padded row pitch for fp8 y tiles (DoubleRow j-step %16==0)
PR = SR + 2      # padded rows per stripe
import os
DWS = int(os.environ.get("DWS", "128"))  # depthwise col split: DVE [0:DWS], Pool [DWS:W]


def host_prep(kv_w, kv_dw_w, q_w, q_dw_w, proj_w, temperature):
    """Host-side weight transforms (all tiny). Returns dict of extra device inputs."""
    import ml_dtypes
    F8NP = ml_dtypes.float8_e4m3
    kv_w = kv_w.astype(np.float64)
    q_w = q_w.astype(np.float64)
    q_dw_w = q_dw_w.astype(np.float64)
    proj_w = proj_w.astype(np.float64)
    # kv 1x1: lhsT = W^T [c_in, c_out=2C]
    wkvT = kv_w[:, :, 0, 0].T.copy()  # [192, 384]
    # fused dense conv: W_eff[o, j, dy, dx] = sum_i q_dw_w[o,i,dy,dx] * q_w[i,j]
    weff = np.einsum("oiyx,ij->ojyx", q_dw_w, q_w[:, :, 0, 0])  # [192,192,3,3]
    # device layout: weffT[j, tap*192 + o]
    weffT = np.transpose(weff, (1, 2, 3, 0)).reshape(C, 9 * C).copy()
    projT = proj_w[:, :, 0, 0].T.copy()  # [c, o]
    # K-packed pair weights for y ch 128..191: tap idx = (dy+1)*3 + (dx+1)
    lo = weffT[128:192, :].reshape(64, 9, C)
    weffP = np.zeros((128, 6 * C), np.float64)
    for p, dy in enumerate((-1, 0, 1)):       # pairs {(dy,0) lower, (dy,-1) upper}
        weffP[0:64, p * C:(p + 1) * C] = lo[:, (dy + 1) * 3 + 1]
        weffP[64:128, p * C:(p + 1) * C] = lo[:, (dy + 1) * 3 + 0]
    weffP[0:64, 3 * C:4 * C] = lo[:, 5]       # (0,+1) single
    weffP[0:64, 4 * C:5 * C] = lo[:, 8]       # (+1,+1) single
    weffP[0:64, 5 * C:6 * C] = lo[:, 2]       # (-1,+1) single
    # ---- fp8 packed dense-conv weights (scaled x256; q l2norm cancels it) ----
    # wf[ych, tap, oc]
    wf = weffT.reshape(C, 9, C) * 256.0
    tapi = lambda dy, dx: (dy + 1) * 3 + (dx + 1)
    hiP = np.zeros((128, 3, 2, C), np.float64)   # pairs {(-1,dx),(0,dx)} over j
    hiS = np.zeros((128, 3, C), np.float64)      # singles (+1,dx)
    for d, dx in enumerate((-1, 0, 1)):
        hiP[:, d, 0, :] = wf[0:128, tapi(-1, dx), :]
        hiP[:, d, 1, :] = wf[0:128, tapi(0, dx), :]
        hiS[:, d, :] = wf[0:128, tapi(1, dx), :]
    # lo packed pairs matching ypB {lower=(dy,0), upper=(dy,-1)}:
    loP = np.zeros((128, 2, C), np.float64)      # j0=(-1,*), j1=(0,*)
    loP[0:64, 0, :] = wf[128:192, tapi(-1, 0), :]
    loP[64:128, 0, :] = wf[128:192, tapi(-1, -1), :]
    loP[0:64, 1, :] = wf[128:192, tapi(0, 0), :]
    loP[64:128, 1, :] = wf[128:192, tapi(0, -1), :]
    loS = np.zeros((128, C), np.float64)         # single (+1,*): {lower=(+1,0), upper=(+1,-1)}
    loS[0:64, :] = wf[128:192, tapi(1, 0), :]
    loS[64:128, :] = wf[128:192, tapi(1, -1), :]
    xpP = np.zeros((64, 2, C), np.float64)       # dx=+1 pair: j0=(-1,+1), j1=(0,+1)
    xpP[:, 0, :] = wf[128:192, tapi(-1, 1), :]
    xpP[:, 1, :] = wf[128:192, tapi(0, 1), :]
    xpS = wf[128:192, tapi(1, 1), :]             # single (+1,+1) [64, C]
    w8hi = np.concatenate([hiP.reshape(128, 6 * C), hiS.reshape(128, 3 * C)], axis=1)
    w8lo = np.concatenate([loP.reshape(128, 2 * C), loS, ], axis=1)
    w8xp = np.concatenate([xpP.reshape(64, 2 * C), xpS], axis=1)
    # ---- fp8 dw-diag weights for m0 (k hi) on PE: pairs {(-1,dx),(0,dx)} + singles ----
    # kvp0 fp8 scale x8; ddg scale x16 => k scaled x128 (per-channel rk cancels)
    dwm0 = kv_dw_w[0:128, 0].astype(np.float64) * 16.0   # [128, 3, 3]
    ddgP = np.zeros((128, 3, 2, 128), np.float64)
    ddgS = np.zeros((128, 3, 128), np.float64)
    rng128 = np.arange(128)
    for d, dx in enumerate((-1, 0, 1)):
        ddgP[rng128, d, 0, rng128] = dwm0[:, 0, dx + 1]
        ddgP[rng128, d, 1, rng128] = dwm0[:, 1, dx + 1]
        ddgS[rng128, d, rng128] = dwm0[:, 2, dx + 1]
    ddg8 = np.concatenate([ddgP.reshape(128, 6 * 128), ddgS.reshape(128, 3 * 128)], axis=1)
    dws = kv_dw_w[:, 0].reshape(2 * C, 9).copy()  # [384, 9]
    tau = np.repeat(np.asarray(temperature, np.float64).reshape(HEADS), CH) * math.log(CH)
    m = np.full((96, 96), -1e9, np.float32)
    m[0:48, 0:48] = 0.0
    m[48:96, 48:96] = 0.0
    return {
        "bmask": m,
        "wkvT": wkvT.astype(np.float32),
        "w8hi": w8hi.astype(F8NP),
        "w8lo": w8lo.astype(F8NP),
        "w8xp": w8xp.astype(F8NP),
        "ddg8": ddg8.astype(F8NP),
        "projT": projT.astype(np.float32),
        "dws": dws.astype(np.float32),
        "tau": tau.reshape(C, 1).astype(np.float32),
    }


def build(H=128, debug=False, reps=1, kv_f32r=False):
    """Build + compile the per-core program. H = image height (rows).

    reps > 1 repeats the whole per-call body back-to-back inside one NEFF;
    used by the test harness to measure steady-state per-iteration device
    time without per-dispatch overhead."""
    HW = H * W
    NS = H // SR  # stripes
    NCK = HW // 512  # output chunks

    XDT = F32R if kv_f32r else F32
    nc = bacc.Bacc("TRN2", target_bir_lowering=False, debug=debug,
                   enable_asserts=False, num_devices=1)
    x = nc.dram_tensor("x", [C, HW], XDT, kind="ExternalInput").ap()
    y = nc.dram_tensor("y", [C, HW], F32, kind="ExternalInput").ap()
    wkvT = nc.dram_tensor("wkvT", [C, 2 * C], XDT, kind="ExternalInput").ap()
    w8hi = nc.dram_tensor("w8hi", [128, 9 * C], F8, kind="ExternalInput").ap()
    w8lo = nc.dram_tensor("w8lo", [128, 3 * C], F8, kind="ExternalInput").ap()
    w8xp = nc.dram_tensor("w8xp", [64, 3 * C], F8, kind="ExternalInput").ap()
    ddg8 = nc.dram_tensor("ddg8", [128, 9 * 128], F8, kind="ExternalInput").ap()
    projT = nc.dram_tensor("projT", [C, C], F32, kind="ExternalInput").ap()
    dws = nc.dram_tensor("dws", [2 * C, 9], F32, kind="ExternalInput").ap()
    tau = nc.dram_tensor("tau", [C, 1], F32, kind="ExternalInput").ap()
    out = nc.dram_tensor("out", [C, HW], F32, kind="ExternalOutput").ap()
    bmask = nc.dram_tensor("bmask", [96, 96], F32, kind="ExternalInput").ap()
    vscrM = nc.dram_tensor("vscrM", [64, HW], BF16, kind="Internal").ap()  # v ch 0..63 spill
    rscr = nc.dram_tensor("rscr", [1, C], F32, kind="Internal").ap()
    zpad = nc.dram_tensor("zpad", [128, W], XDT, kind="Internal").ap()  # zero halo source

    taps = [(dy, dx) for dy in (-1, 0, 1) for dx in (-1, 0, 1)]

    with tile.TileContext(nc) as tc:
      for _rep in range(reps):
        with ExitStack() as ctx:
            wp = ctx.enter_context(tc.tile_pool(name="wp", bufs=1))       # persistent sbuf
            pp_ctx = ExitStack()
            pp = pp_ctx.enter_context(tc.tile_pool(name="pp", bufs=1, space="PSUM"))  # psum: raw/gram, freed after phase 2

            # ---- weights to sbuf (+ bf16 casts where needed) ----
            stg_ctx = ExitStack()
            stg = stg_ctx.enter_context(tc.tile_pool(name="stg", bufs=2))

            def load_cast(ap_dram, p, f, nm):
                t32 = stg.tile([p, f], F32, tag="ldstage", name=f"stage_{nm}")
                nc.sync.dma_start(t32[:], ap_dram)
                tb = wp.tile([p, f], BF16, tag=nm, name=nm)
                nc.gpsimd.tensor_copy(tb[:], t32[:])
                return tb

            if kv_f32r:
                wkv_hi = wp.tile([128, 2 * C], F32R, name="wkv_hi")
                nc.sync.dma_start(wkv_hi[:], wkvT[0:128, :])
                wkv_lo = wp.tile([64, 2 * C], F32R, name="wkv_lo")
                nc.sync.dma_start(wkv_lo[:], wkvT[128:192, :])
            else:
                wkv_hi = load_cast(wkvT[0:128, :], 128, 2 * C, "wkv_hi")
                wkv_lo = load_cast(wkvT[128:192, :], 64, 2 * C, "wkv_lo")
            zt = wp.tile([128, W], F32, name="zt")
            nc.vector.memset(zt[:], 0.0)
            nc.sync.dma_start(zpad[:], zt[:].bitcast(XDT))
            w8hi_t = wp.tile([128, 9 * C], F8, name="w8hi_t")
            nc.sync.dma_start(w8hi_t[:], w8hi)
            w8lo_t = wp.tile([128, 3 * C], F8, name="w8lo_t")
            nc.sync.dma_start(w8lo_t[:], w8lo)
            w8xp_t = wp.tile([64, 3 * C], F8, name="w8xp_t")
            nc.sync.dma_start(w8xp_t[:], w8xp)
            ddg8_t = wp.tile([128, 9 * 128], F8, name="ddg8_t")
            nc.sync.dma_start(ddg8_t[:], ddg8)
            prA = load_cast(projT[0:96, :], 96, C, "prA")
            prB = load_cast(projT[96:192, :], 96, C, "prB")
            dws_t = wp.tile([128, 9 * 3], F32)  # 3 chunks side by side: [:,9m+t]
            for m in range(3):
                nc.sync.dma_start(dws_t[0:128, 9 * m:9 * m + 9], dws[128 * m:128 * m + 128, :])
            tauA = wp.tile([128, 1], F32)
            nc.sync.dma_start(tauA[:], tau[0:128, :])
            tauB = wp.tile([64, 1], F32)
            nc.sync.dma_start(tauB[:], tau[128:192, :])
            identF = wp.tile([128, 128], F32)
            make_identity(nc, identF[:])
            stg_ctx.close()

            # ssq accumulator slots (per stripe), fp32
            ssqA = wp.tile([128, NS], F32)   # k ch 0..127
            ssqB = wp.tile([64, NS], F32)    # k ch 128..191

            # persistent v ch 64..191 in SBUF (bf16); ch 0..63 spills to vscrM
            vsbB = wp.tile([128, HW], BF16, name="vsbB")

            # persistent psum: attn raw blocks + q gram packed into one bank.
            # Zeroed explicitly; the accumulating matmuls use start=False so
            # bank-granular start-zeroing can't clobber neighbours.
            ppbig = pp.tile([128, 384], F32, name="ppbig")
            nc.vector.memset(ppbig[:], 0.0)
            raw01 = ppbig[0:96, 0:96]
            raw23 = ppbig[0:96, 96:192]
            gq_hi = ppbig[0:128, 192:320]
            gq_lo = ppbig[0:64, 320:384]

            with ExitStack() as sctx:
                sp = sctx.enter_context(tc.tile_pool(name="sp", bufs=2))      # stripe transients
                sp1 = sctx.enter_context(tc.tile_pool(name="sp1", bufs=1))    # single-buffered stages
                qp = sctx.enter_context(tc.tile_pool(name="qp", bufs=4))
                kvpool = sctx.enter_context(tc.tile_pool(name="kvp", bufs=2))
                pk = sctx.enter_context(tc.tile_pool(name="pk", bufs=3, space="PSUM"))
                pq = sctx.enter_context(tc.tile_pool(name="pq", bufs=3, space="PSUM"))

                # per-stripe produce: stage inputs, kv conv, casts/packs, dw, kt
                def produce(s):
                    r_lo = SR * s - 1           # first (halo) image row
                    n_lo = r_lo * W

                    # ---- x DMA (f32r, fed to PE directly) ----
                    xA32 = sp.tile([128, PR * W], XDT, tag="xA32", name="xA32")
                    xB32 = sp.tile([64, PR * W], XDT, tag="xB32", name="xB32")
                    for t, p0, p in ((xA32, 0, 128), (xB32, 128, 64)):
                        if s == 0:
                            nc.sync.dma_start(t[:, 0:W], zpad[0:p, :])
                            nc.sync.dma_start(t[:, W:], x[p0:p0 + p, 0:(PR - 1) * W])
                        elif s == NS - 1:
                            nc.sync.dma_start(t[:, (PR - 1) * W:], zpad[0:p, :])
                            nc.sync.dma_start(t[:, 0:(PR - 1) * W], x[p0:p0 + p, n_lo:n_lo + (PR - 1) * W])
                        else:
                            nc.sync.dma_start(t[:], x[p0:p0 + p, n_lo:n_lo + PR * W])

                    if not kv_f32r:
                        xbA = sp.tile([128, PR * W], BF16, tag="xbA")
                        nc.gpsimd.tensor_copy(xbA[:], xA32[:])
                        xbB = sp.tile([64, PR * W], BF16, tag="xbB")
                        nc.gpsimd.tensor_copy(xbB[:], xB32[:])
                        kv_in_A, kv_in_B = xbA, xbB
                    else:
                        kv_in_A, kv_in_B = xA32, xB32

                    # ---- y DMA (f32) ----
                    yA32 = sp1.tile([128, PR * W], F32, tag="yA32", name="yA32")
                    if s == 0:
                        nc.gpsimd.memset(yA32[:, 0:W], 0.0)
                        nc.sync.dma_start(yA32[:, W:], y[0:128, 0:(PR - 1) * W])
                    elif s == NS - 1:
                        nc.gpsimd.memset(yA32[:, (PR - 1) * W:], 0.0)
                        nc.sync.dma_start(yA32[:, 0:(PR - 1) * W], y[0:128, n_lo:n_lo + (PR - 1) * W])
                    else:
                        nc.sync.dma_start(yA32[:], y[0:128, n_lo:n_lo + PR * W])
                    # rows 16s-2 .. 16s+16 (19 rows) for the row-shifted packs
                    yB19 = sp1.tile([64, (PR + 1) * W], F32, tag="yB19", name="yB19")
                    lo2 = (SR * s - 2) * W
                    a, b = 0, (PR + 1) * W
                    if s == 0:
                        nc.gpsimd.memset(yB19[:, 0:2 * W], 0.0)
                        a = 2 * W
                    if s == NS - 1:
                        nc.gpsimd.memset(yB19[:, (PR + 1 - 1) * W:], 0.0)
                        b = (PR + 1 - 1) * W
                    nc.sync.dma_start(yB19[:, a:b], y[128:192, lo2 + a:lo2 + b])

                    # ---- y casts (DVE) ----
                    ypA = sp.tile([128, PR, PWQ], F8, tag="ypA", name="ypA")
                    nc.gpsimd.memset(ypA[:, :, 0:1], 0.0)
                    nc.gpsimd.memset(ypA[:, :, 1 + W:3 + W], 0.0)
                    nc.scalar.copy(
                        ypA[:, :, 1:1 + W],
                        yA32[:].rearrange("p (a b) -> p a b", b=W))
                    ybB = sp1.tile([64, PR + 1, W], F8, tag="ybB", name="ybB")
                    nc.scalar.copy(ybB[:], yB19[:].rearrange("p (a b) -> p a b", b=W))

                    # ---- packed y-lo tile via local DMA dups ----
                    # ypB: lower = ybB @col1 (taps (dy,0) and, read at col+2, (dy,+1)),
                    #      upper = ybB @col2 (taps (dy,-1))
                    ypB = sp.tile([128, PR + 1, PWQ], F8, tag="ypB", name="ypB")
                    nc.gpsimd.memset(ypB[0:64, :, 0:1], 0.0)
                    nc.gpsimd.memset(ypB[0:64, :, 1 + W:3 + W], 0.0)
                    nc.gpsimd.memset(ypB[64:128, :, 0:2], 0.0)
                    nc.gpsimd.memset(ypB[64:128, :, 2 + W:3 + W], 0.0)
                    nc.gpsimd.dma_start(ypB[0:64, :, 1:1 + W], ybB[:])
                    nc.gpsimd.dma_start(ypB[64:128, :, 2:2 + W], ybB[:])

                    # ---- kv 1x1 conv (fp32r) -> padded kvp chunks ----
                    kvp = []
                    for m in range(3):
                        if m == 0:
                            kvt = kvpool.tile([128, PR, PWQ], F8, tag=f"kvp{m}")
                            nc.gpsimd.memset(kvt[:, :, 0:1], 0.0)
                            nc.gpsimd.memset(kvt[:, :, 1 + W:3 + W], 0.0)
                        else:
                            kvt = kvpool.tile([128, PR, PW], BF16, tag=f"kvp{m}")
                            nc.gpsimd.memset(kvt[:, :, 0:1], 0.0)
                            nc.gpsimd.memset(kvt[:, :, PW - 1:PW], 0.0)
                        kvp.append(kvt)
                        lhs_hi = wkv_hi[:, 128 * m:128 * m + 128]
                        lhs_lo = wkv_lo[:, 128 * m:128 * m + 128]
                        for j in range(0, PR * W, 512):
                            w_ = min(512, PR * W - j)
                            pst = pk.tile([128, 512], F32, tag="pkv", name="pkv")
                            ps = pst[:, 0:w_]
                            nc.tensor.matmul(ps, lhs_hi, kv_in_A[:, j:j + w_], start=True, stop=False)
                            nc.tensor.matmul(ps, lhs_lo, kv_in_B[:, j:j + w_], start=False, stop=True)
                            if m == 0:
                                nc.scalar.activation(
                                    kvt[:, j // W:j // W + w_ // W, 1:1 + W],
                                    ps.rearrange("p (a b) -> p a b", b=W),
                                    AF.Copy, scale=8.0)
                            else:
                                nc.scalar.copy(
                                    kvt[:, j // W:j // W + w_ // W, 1:1 + W],
                                    ps.rearrange("p (a b) -> p a b", b=W))

                    # ---- depthwise 3x3: m0 on PE (fp8 dwdiag, DoubleRow dy-pairs),
                    #      m1/m2 elementwise on DVE/Pool ----
                    kA = sp.tile([128, SR, W], BF16, tag="kA")
                    for jr in range(0, SR, 4):
                        pdw = pk.tile([128, 512], F32, tag="pkv", name="pdw")
                        for d, dx in enumerate((-1, 0, 1)):
                            # rhs [K, j=2(dy -1/0, step PWQ), row=4, col=W]
                            b4 = kvp[0][:, jr:jr + 4, 1 + dx:1 + dx + W].unsqueeze(1)
                            b4 = b4.broadcast_to([128, 2, 4, W])
                            v4 = b4.ap
                            v4[1] = [PWQ, 2]
                            b4.ap = v4
                            lhsp = ddg8_t[:, d * 256:(d + 1) * 256].rearrange(
                                "p (j m) -> p j m", j=2)
                            nc.tensor.matmul(pdw[:], lhsp, b4, start=(d == 0), stop=False,
                                             perf_mode=PM.DoubleRow)
                        for d, dx in enumerate((-1, 0, 1)):
                            rhs1 = kvp[0][:, jr + 2:jr + 6, 1 + dx:1 + dx + W]
                            lhs1 = ddg8_t[:, 768 + d * 128:768 + (d + 1) * 128]
                            nc.tensor.matmul(pdw[:], lhs1, rhs1, start=False, stop=(d == 2))
                        nc.scalar.copy(kA[:, jr:jr + 4, :],
                                       pdw[:].rearrange("p (a b) -> p a b", b=W))
                    kvmid = sp.tile([128, SR, W], BF16, tag="kvmid")
                    vBv = vsbB[:, SR * s * W:(SR * s + SR) * W].rearrange("p (a b) -> p a b", b=W)
                    douts = [None, kvmid[:, :, :], vBv]
                    for m in range(1, 3):
                        dst = douts[m]
                        tmp = sp.tile([128, SR, W], BF16, tag="dwtmp", name="dwtmp")
                        for ti, (dy, dx) in enumerate(taps):
                            sc = dws_t[:, 9 * m + ti:9 * m + ti + 1]
                            src = kvp[m][:, 1 + dy:1 + SR + dy, 1 + dx:1 + dx + W]
                            for eng, c0, c1 in ((nc.vector, 0, DWS), (nc.gpsimd, DWS, W)):
                                if c0 >= c1:
                                    continue
                                d = dst[:, :, c0:c1]
                                sl = src[:, :, c0:c1]
                                if ti == 0:
                                    eng.tensor_scalar_mul(d, sl, sc)
                                else:
                                    t_ = tmp[:, :, c0:c1]
                                    eng.tensor_scalar_mul(t_, sl, sc)
                                    eng.tensor_add(d, d, t_)

                    # ---- k transpose: kt[p, r, c] = k[c, r*W + p] ----
                    kt = sp.tile([128, SR, C], BF16, tag="kt")
                    nc.scalar.dma_start_transpose(kt[:, :, 0:128], kA[:].rearrange("p a b -> p (a b)"))
                    nc.scalar.dma_start_transpose(kt[:, :, 128:192], kvmid[0:64].rearrange("p a b -> p (a b)"))

                    # ---- v ch 0..63 spill to DRAM ----
                    nsl = slice(SR * s * W, SR * s * W + SR * W)
                    nc.scalar.dma_start(vscrM[:, nsl], kvmid[64:128].rearrange("p a b -> p (a b)"))

                    return ypA, ypB, kt, kA, kvmid

                def produce_tail(s, st):
                    # ssq_k (in-place square; emitted after consume so the Act
                    # queue isn't blocked between kvp and qsb copies)
                    _, _, _, kA, kvmid = st
                    nc.scalar.activation(kA[:], kA[:], AF.Square, accum_out=ssqA[:, s:s + 1])
                    nc.scalar.activation(kvmid[0:64], kvmid[0:64], AF.Square, accum_out=ssqB[:, s:s + 1])

                # ---- dense conv (fused q) + attn/gram accumulation for stripe s ----
                def consume(s, st):
                    ypA, ypB, kt = st[0], st[1], st[2]

                    def attn_mms(qsb, r, kt_):
                        l = (s == NS - 1 and r == SR - 1)
                        nc.tensor.matmul(gq_hi, qsb[:, 0:128], qsb[:, 0:128], start=False, stop=l,
                                         skip_group_check=True)
                        nc.tensor.matmul(gq_lo, qsb[:, 128:192], qsb[:, 128:192], start=False, stop=l,
                                         skip_group_check=True)
                        nc.tensor.matmul(raw01, qsb[:, 0:96], kt_[:, r, 0:96], start=False, stop=l,
                                         skip_group_check=True)
                        nc.tensor.matmul(raw23, qsb[:, 96:192], kt_[:, r, 96:192], start=False, stop=l,
                                         skip_group_check=True)

                    pends = []
                    for r in range(SR):
                        psq = pq.tile([128, C], F32, tag="psq")
                        # y-hi: 3 dy-pair DoubleRows + 3 dy=+1 singles
                        for d, dx in enumerate((-1, 0, 1)):
                            lhs = ypA[:, r:r + 2, 1 + dx:1 + dx + W]
                            rhs = w8hi_t[:, d * 2 * C:(d * 2 + 2) * C].rearrange(
                                "p (j n) -> p j n", j=2)
                            nc.tensor.matmul(psq, lhs, rhs, start=(d == 0), stop=False,
                                             perf_mode=PM.DoubleRow)
                        for d, dx in enumerate((-1, 0, 1)):
                            nc.tensor.matmul(psq, ypA[:, r + 2, 1 + dx:1 + dx + W],
                                             w8hi_t[:, (6 + d) * C:(7 + d) * C],
                                             start=False, stop=False)
                        # y-lo packed {lower=(dy,0), upper=(dy,-1)}: dy-pair DR + single
                        rhs = w8lo_t[:, 0:2 * C].rearrange("p (j n) -> p j n", j=2)
                        nc.tensor.matmul(psq, ypB[:, r + 1:r + 3, 1:1 + W], rhs,
                                         start=False, stop=False, perf_mode=PM.DoubleRow)
                        nc.tensor.matmul(psq, ypB[:, r + 3, 1:1 + W], w8lo_t[:, 2 * C:3 * C],
                                         start=False, stop=False)
                        # y-lo dx=+1 (reads @col3): dy-pair DR (K=64) + single
                        rhs = w8xp_t[:, 0:2 * C].rearrange("p (j n) -> p j n", j=2)
                        nc.tensor.matmul(psq, ypB[0:64, r + 1:r + 3, 3:3 + W], rhs,
                                         start=False, stop=False, perf_mode=PM.DoubleRow)
                        nc.tensor.matmul(psq, ypB[0:64, r + 3, 3:3 + W], w8xp_t[:, 2 * C:3 * C],
                                         start=False, stop=True)
                        qsb = qp.tile([128, C], BF16, tag="qsb")
                        nc.scalar.copy(qsb[:], psq[:])
                        pends.append((qsb, r, kt))
                        if len(pends) > 2:
                            attn_mms(*pends.pop(0))
                        if r == SR - 1:
                            for p_ in pends:
                                attn_mms(*p_)
                            pends = []

                # software pipeline: produce stripe i, consume stripe i-1
                prev = None
                for i in range(NS + 1):
                    cur = produce(i) if i < NS else None
                    if prev is not None:
                        consume(i - 1, prev)
                        produce_tail(i - 1, prev)
                    prev = cur

            # ================= phase 2: softmax + MT =================
            mt_M = wp.tile([64, C], BF16, name="mt_M")    # rows = v ch 0..63
            mt_B = wp.tile([128, C], BF16, name="mt_B")   # rows = v ch 64..191
            with ExitStack() as sctx2:
                s2 = sctx2.enter_context(tc.tile_pool(name="s2", bufs=1))
                p2 = sctx2.enter_context(tc.tile_pool(name="p2", bufs=1, space="PSUM"))

                # ssq_q from gram diagonals
                scr2 = s2.tile([128, 128], F32)
                ssqqA = s2.tile([128, 1], F32)
                nc.vector.scalar_tensor_tensor(scr2[:], gq_hi, 1.0, identF[:],
                                               AL.mult, AL.mult, accum_out=ssqqA[:])
                scr2b = s2.tile([64, 64], F32)
                ssqqB = s2.tile([64, 1], F32)
                nc.vector.scalar_tensor_tensor(scr2b[:], gq_lo, 1.0, identF[0:64, 0:64],
                                               AL.mult, AL.mult, accum_out=ssqqB[:])
                # ssq_k totals
                sskA = s2.tile([128, 1], F32)
                nc.vector.reduce_sum(sskA[:], ssqA[:], axis=mybir.AxisListType.X)
                sskB = s2.tile([64, 1], F32)
                nc.vector.reduce_sum(sskB[:], ssqB[:], axis=mybir.AxisListType.X)

                def rsqrt(dst, src):
                    nc.vector.reciprocal(dst, src)
                    nc.scalar.activation(dst, dst, AF.Sqrt)

                rqA = s2.tile([128, 1], F32, name="rqA")
                rsqrt(rqA[:], ssqqA[:])
                rqB = s2.tile([64, 1], F32, name="rqB")
                rsqrt(rqB[:], ssqqB[:])
                rkA = s2.tile([128, 1], F32, name="rkA")
                rsqrt(rkA[:], sskA[:])
                rkB = s2.tile([64, 1], F32, name="rkB")
                rsqrt(rkB[:], sskB[:])
                # rq * tau
                nc.vector.tensor_mul(rqA[:], rqA[:], tauA[:])
                nc.vector.tensor_mul(rqB[:], rqB[:], tauB[:])

                # rk rows [1, 192] via DRAM bounce (SBUF partition-transpose DMA is illegal)
                nc.scalar.dma_start(rscr[0:1, 0:128].rearrange("a b -> b a"), rkA[:])
                nc.scalar.dma_start(rscr[0:1, 128:192].rearrange("a b -> b a"), rkB[:])
                rkrow = s2.tile([1, 192], F32)
                nc.scalar.dma_start(rkrow[:], rscr)
                rkrow_b = s2.tile([1, 192], BF16)
                nc.vector.tensor_copy(rkrow_b[:], rkrow[:])
                ones1 = s2.tile([1, 96], BF16)
                nc.vector.memset(ones1[:], 1.0)
                rkb01p = p2.tile([96, 96], F32)
                nc.tensor.matmul(rkb01p[:], ones1[:], rkrow_b[0:1, 0:96], start=True, stop=True)
                rkb23p = p2.tile([96, 96], F32)
                nc.tensor.matmul(rkb23p[:], ones1[:], rkrow_b[0:1, 96:192], start=True, stop=True)

                # logits = raw * (rq*tau) * rk
                l01 = s2.tile([96, 96], F32)
                nc.scalar.activation(l01[:], raw01, AF.Copy, scale=rqA[0:96, :])
                nc.vector.tensor_mul(l01[:], l01[:], rkb01p[:])
                l23 = s2.tile([96, 96], F32)
                rq23 = s2.tile([96, 1], F32)
                nc.scalar.dma_start(rq23[0:32, :], rqA[96:128, :])
                nc.scalar.dma_start(rq23[32:96, :], rqB[:])
                nc.scalar.activation(l23[:], raw23, AF.Copy, scale=rq23[:])
                nc.vector.tensor_mul(l23[:], l23[:], rkb23p[:])

                # softmax per head-pair with additive block mask -> blockdiag bd (bf16)
                msk = s2.tile([96, 96], F32)
                nc.sync.dma_start(msk[:], bmask)
                bd01 = s2.tile([96, 96], BF16)
                bd23 = s2.tile([96, 96], BF16)
                for hb, (lt, bd) in enumerate(((l01, bd01), (l23, bd23))):
                    nc.vector.tensor_add(lt[:], lt[:], msk[:])
                    mx = s2.tile([96, 1], F32, tag=f"mx{hb}", name=f"mx{hb}")
                    nc.vector.reduce_max(mx[:], lt[:], axis=mybir.AxisListType.X)
                    nc.vector.tensor_scalar_mul(mx[:], mx[:], -1.0)
                    ex = s2.tile([96, 96], F32, tag=f"ex{hb}", name=f"ex{hb}")
                    rs = s2.tile([96, 1], F32, tag=f"rs{hb}", name=f"rs{hb}")
                    nc.scalar.activation(ex[:], lt[:], AF.Exp, bias=mx[:], accum_out=rs[:])
                    nc.vector.reciprocal(rs[:], rs[:])
                    nc.vector.tensor_scalar_mul(bd[:], ex[:], rs[:])

                # MT[d, o] = sum_c attn[c, d] * projT[c, o], laid out to match v tiles
                mtM_p = p2.tile([64, C], F32)
                nc.tensor.matmul(mtM_p[:], bd01[:, 0:64], prA[:], start=True, stop=True)
                mtB_p = p2.tile([128, C], F32)
                nc.tensor.matmul(mtB_p[0:32, :], bd01[:, 64:96], prA[:], start=True, stop=True)
                nc.tensor.matmul(mtB_p[32:64, :], bd23[:, 0:32], prB[:], start=True, stop=True)
                nc.tensor.matmul(mtB_p[64:128, :], bd23[:, 32:96], prB[:], start=True, stop=True)
                nc.scalar.copy(mt_M[:], mtM_p[:])
                nc.scalar.copy(mt_B[:], mtB_p[:])
            pp_ctx.close()

            # ================= phase 3: out = MT.T @ v from SBUF =================
            with ExitStack() as sctx3:
                s3 = sctx3.enter_context(tc.tile_pool(name="s3", bufs=3))
                p3 = sctx3.enter_context(tc.tile_pool(name="p3", bufs=3, space="PSUM"))
                for j in range(NCK):
                    nsl = slice(512 * j, 512 * j + 512)
                    vM = s3.tile([64, 512], BF16, tag="vM")
                    nc.sync.dma_start(vM[:], vscrM[:, nsl])
                    f1 = p3.tile([128, 512], F32, tag="f1")
                    nc.tensor.matmul(f1[:], mt_M[:, 0:128], vM[:], start=True, stop=False)
                    nc.tensor.matmul(f1[:], mt_B[:, 0:128], vsbB[:, nsl], start=False, stop=True)
                    f2 = p3.tile([64, 512], F32, tag="f2")
                    nc.tensor.matmul(f2[:], mt_M[:, 128:192], vM[:], start=True, stop=False)
                    nc.tensor.matmul(f2[:], mt_B[:, 128:192], vsbB[:, nsl], start=False, stop=True)
                    o1 = s3.tile([128, 512], F32, tag="o1")
                    nc.vector.tensor_copy(o1[:], f1[:])
                    o2 = s3.tile([64, 512], F32, tag="o2")
                    nc.scalar.copy(o2[:], f2[:])
                    nc.sync.dma_start(out[0:128, nsl], o1[:])
                    nc.sync.dma_start(out[128:192, nsl], o2[:])

    nc.compile()
    return nc


# ======================= harness entry point =======================
B = 8
H = 128
_NC = None


def _get_nc():
    global _NC
    if _NC is None:
        _NC = build(H=H)
    return _NC


def _make_in_maps(inputs):
    x = np.ascontiguousarray(inputs["x"], np.float32)
    y = np.ascontiguousarray(inputs["y"], np.float32)
    prep = host_prep(inputs["kv_w"], inputs["kv_dw_w"], inputs["q_w"],
                     inputs["q_dw_w"], inputs["proj_w"], inputs["temperature"])
    maps = []
    for b in range(B):
        m = {"x": x[b].reshape(C, H * W), "y": y[b].reshape(C, H * W)}
        m.update(prep)
        maps.append(m)
    return maps


def _run(inputs, trace=False, trace_kwargs=None):
    from concourse.bass_utils import run_bass_kernel_spmd
    nc = _get_nc()
    res = run_bass_kernel_spmd(nc, _make_in_maps(inputs), core_ids=list(range(B)),
                               trace=trace, trace_kwargs=trace_kwargs or {})
    out = np.stack([np.asarray(res.results[b]["out"], np.float32).reshape(C, H, W)
                    for b in range(B)])
    return out, res


def kernel(**inputs) -> np.ndarray:
    out, _ = _run(inputs, trace=False)
    return out


# revision 12
# speedup vs baseline: 1.0531x; 1.0531x over previous
import math
from contextlib import ExitStack

import numpy as np

import concourse.bass as bass
import concourse.tile as tile
from concourse import bacc, mybir
from concourse.masks import make_identity

F32 = mybir.dt.float32
F32R = mybir.dt.float32r
BF16 = mybir.dt.bfloat16
F8 = mybir.dt.float8e4
PM = mybir.MatmulPerfMode
AL = mybir.AluOpType
AF = mybir.ActivationFunctionType

C = 192          # channels
HEADS = 4
CH = C // HEADS  # 48
W = 128          # image width
SR = 16          # rows per stripe
PW = W + 2       # padded width (ypA)
PWQ = 144        # padded row pitch for fp8 y tiles (DoubleRow j-step %16==0)
PR = SR + 2      # padded rows per stripe
import os
DWS = int(os.environ.get("DWS", "128"))  # depthwise col split: DVE [0:DWS], Pool [DWS:W]


def host_prep(kv_w, kv_dw_w, q_w, q_dw_w, proj_w, temperature):
    """Host-side weight transforms (all tiny). Returns dict of extra device inputs."""
    import ml_dtypes
    F8NP = ml_dtypes.float8_e4m3
    kv_w = kv_w.astype(np.float64)
    q_w = q_w.astype(np.float64)
    q_dw_w = q_dw_w.astype(np.float64)
    proj_w = proj_w.astype(np.float64)
    # kv 1x1: lhsT = W^T [c_in, c_out=2C]
    wkvT = kv_w[:, :, 0, 0].T.copy()  # [192, 384]
    # fused dense conv: W_eff[o, j, dy, dx] = sum_i q_dw_w[o,i,dy,dx] * q_w[i,j]
    weff = np.einsum("oiyx,ij->ojyx", q_dw_w, q_w[:, :, 0, 0])  # [192,192,3,3]
    # device layout: weffT[j, tap*192 + o]
    weffT = np.transpose(weff, (1, 2, 3, 0)).reshape(C, 9 * C).copy()
    projT = proj_w[:, :, 0, 0].T.copy()  # [c, o]
    # K-packed pair weights for y ch 128..191: tap idx = (dy+1)*3 + (dx+1)
    lo = weffT[128:192, :].reshape(64, 9, C)
    weffP = np.zeros((128, 6 * C), np.float64)
    for p, dy in enumerate((-1, 0, 1)):       # pairs {(dy,0) lower, (dy,-1) upper}
        weffP[0:64, p * C:(p + 1) * C] = lo[:, (dy + 1) * 3 + 1]
        weffP[64:128, p * C:(p + 1) * C] = lo[:, (dy + 1) * 3 + 0]
    weffP[0:64, 3 * C:4 * C] = lo[:, 5]       # (0,+1) single
    weffP[0:64, 4 * C:5 * C] = lo[:, 8]       # (+1,+1) single
    weffP[0:64, 5 * C:6 * C] = lo[:, 2]       # (-1,+1) single
    # ---- fp8 packed dense-conv weights (scaled x256; q l2norm cancels it) ----
    # wf[ych, tap, oc]
    wf = weffT.reshape(C, 9, C) * 256.0
    tapi = lambda dy, dx: (dy + 1) * 3 + (dx + 1)
    hiP = np.zeros((128, 3, 2, C), np.float64)   # pairs {(-1,dx),(0,dx)} over j
    hiS = np.zeros((128, 3, C), np.float64)      # singles (+1,dx)
    for d, dx in enumerate((-1, 0, 1)):
        hiP[:, d, 0, :] = wf[0:128, tapi(-1, dx), :]
        hiP[:, d, 1, :] = wf[0:128, tapi(0, dx), :]
        hiS[:, d, :] = wf[0:128, tapi(1, dx), :]
    # lo packed pairs matching ypB {lower=(dy,0), upper=(dy,-1)}:
    loP = np.zeros((128, 2, C), np.float64)      # j0=(-1,*), j1=(0,*)
    loP[0:64, 0, :] = wf[128:192, tapi(-1, 0), :]
    loP[64:128, 0, :] = wf[128:192, tapi(-1, -1), :]
    loP[0:64, 1, :] = wf[128:192, tapi(0, 0), :]
    loP[64:128, 1, :] = wf[128:192, tapi(0, -1), :]
    loS = np.zeros((128, C), np.float64)         # single (+1,*): {lower=(+1,0), upper=(+1,-1)}
    loS[0:64, :] = wf[128:192, tapi(1, 0), :]
    loS[64:128, :] = wf[128:192, tapi(1, -1), :]
    xpP = np.zeros((64, 2, C), np.float64)       # dx=+1 pair: j0=(-1,+1), j1=(0,+1)
    xpP[:, 0, :] = wf[128:192, tapi(-1, 1), :]
    xpP[:, 1, :] = wf[128:192, tapi(0, 1), :]
    xpS = wf[128:192, tapi(1, 1), :]             # single (+1,+1) [64, C]
    w8hi = np.concatenate([hiP.reshape(128, 6 * C), hiS.reshape(128, 3 * C)], axis=1)
    w8lo = np.concatenate([loP.reshape(128, 2 * C), loS, ], axis=1)
    w8xp = np.concatenate([xpP.reshape(64, 2 * C), xpS], axis=1)
    # ---- fp8 dw-diag weights for m0 (k hi) on PE: pairs {(-1,dx),(0,dx)} + singles ----
    # kvp0 fp8 scale x8; ddg scale x16 => k scaled x128 (per-channel rk cancels)
    dwm0 = kv_dw_w[0:128, 0].astype(np.float64) * 16.0   # [128, 3, 3]
    ddgP = np.zeros((128, 3, 2, 128), np.float64)
    ddgS = np.zeros((128, 3, 128), np.float64)
    rng128 = np.arange(128)
    for d, dx in enumerate((-1, 0, 1)):
        ddgP[rng128, d, 0, rng128] = dwm0[:, 0, dx + 1]
        ddgP[rng128, d, 1, rng128] = dwm0[:, 1, dx + 1]
        ddgS[rng128, d, rng128] = dwm0[:, 2, dx + 1]
    ddg8 = np.concatenate([ddgP.reshape(128, 6 * 128), ddgS.reshape(128, 3 * 128)], axis=1)
    dws = kv_dw_w[:, 0].reshape(2 * C, 9).copy()  # [384, 9]
    tau = np.repeat(np.asarray(temperature, np.float64).reshape(HEADS), CH) * math.log(CH)
    m = np.full((96, 96), -1e9, np.float32)
    m[0:48, 0:48] = 0.0
    m[48:96, 48:96] = 0.0
    return {
        "bmask": m,
        "wkvT": wkvT.astype(np.float32),
        "w8hi": w8hi.astype(F8NP),
        "w8lo": w8lo.astype(F8NP),
        "w8xp": w8xp.astype(F8NP),
        "ddg8": ddg8.astype(F8NP),
        "projT": projT.astype(np.float32),
        "dws": dws.astype(np.float32),
        "tau": tau.reshape(C, 1).astype(np.float32),
    }


def build(H=128, debug=False, reps=1, kv_f32r=False):
    """Build + compile the per-core program. H = image height (rows).

    reps > 1 repeats the whole per-call body back-to-back inside one NEFF;
    used by the test harness to measure steady-state per-iteration device
    time without per-dispatch overhead."""
    HW = H * W
    NS = H // SR  # stripes
    NCK = HW // 512  # output chunks

    XDT = F32R if kv_f32r else F32
    nc = bacc.Bacc("TRN2", target_bir_lowering=False, debug=debug,
                   enable_asserts=False, num_devices=1)
    x = nc.dram_tensor("x", [C, HW], XDT, kind="ExternalInput").ap()
    y = nc.dram_tensor("y", [C, HW], F32, kind="ExternalInput").ap()
    wkvT = nc.dram_tensor("wkvT", [C, 2 * C], XDT, kind="ExternalInput").ap()
    w8hi = nc.dram_tensor("w8hi", [128, 9 * C], F8, kind="ExternalInput").ap()
    w8lo = nc.dram_tensor("w8lo", [128, 3 * C], F8, kind="ExternalInput").ap()
    w8xp = nc.dram_tensor("w8xp", [64, 3 * C], F8, kind="ExternalInput").ap()
    ddg8 = nc.dram_tensor("ddg8", [128, 9 * 128], F8, kind="ExternalInput").ap()
    projT = nc.dram_tensor("projT", [C, C], F32, kind="ExternalInput").ap()
    dws = nc.dram_tensor("dws", [2 * C, 9], F32, kind="ExternalInput").ap()
    tau = nc.dram_tensor("tau", [C, 1], F32, kind="ExternalInput").ap()
    out = nc.dram_tensor("out", [C, HW], F32, kind="ExternalOutput").ap()
    bmask = nc.dram_tensor("bmask", [96, 96], F32, kind="ExternalInput").ap()
    vscrM = nc.dram_tensor("vscrM", [64, HW], BF16, kind="Internal").ap()  # v ch 0..63 spill
    rscr = nc.dram_tensor("rscr", [1, C], F32, kind="Internal").ap()
    zpad = nc.dram_tensor("zpad", [128, W], XDT, kind="Internal").ap()  # zero halo source

    taps = [(dy, dx) for dy in (-1, 0, 1) for dx in (-1, 0, 1)]

    with tile.TileContext(nc) as tc:
      for _rep in range(reps):
        with ExitStack() as ctx:
            wp = ctx.enter_context(tc.tile_pool(name="wp", bufs=1))       # persistent sbuf
            pp_ctx = ExitStack()
            pp = pp_ctx.enter_context(tc.tile_pool(name="pp", bufs=1, space="PSUM"))  # psum: raw/gram, freed after phase 2

            # ---- weights to sbuf (+ bf16 casts where needed) ----
            stg_ctx = ExitStack()
            stg = stg_ctx.enter_context(tc.tile_pool(name="stg", bufs=2))

            def load_cast(ap_dram, p, f, nm):
                t32 = stg.tile([p, f], F32, tag="ldstage", name=f"stage_{nm}")
                nc.sync.dma_start(t32[:], ap_dram)
                tb = wp.tile([p, f], BF16, tag=nm, name=nm)
                nc.gpsimd.tensor_copy(tb[:], t32[:])
                return tb

            if kv_f32r:
                wkv_hi = wp.tile([128, 2 * C], F32R, name="wkv_hi")
                nc.sync.dma_start(wkv_hi[:], wkvT[0:128, :])
                wkv_lo = wp.tile([64, 2 * C], F32R, name="wkv_lo")
                nc.sync.dma_start(wkv_lo[:], wkvT[128:192, :])
            else:
                wkv_hi = load_cast(wkvT[0:128, :], 128, 2 * C, "wkv_hi")
                wkv_lo = load_cast(wkvT[128:192, :], 64, 2 * C, "wkv_lo")
            zt = wp.tile([128, W], F32, name="zt")
            nc.vector.memset(zt[:], 0.0)
            nc.sync.dma_start(zpad[:], zt[:].bitcast(XDT))
            w8hi_t = wp.tile([128, 9 * C], F8, name="w8hi_t")
            nc.sync.dma_start(w8hi_t[:], w8hi)
            w8lo_t = wp.tile([128, 3 * C], F8, name="w8lo_t")
            nc.sync.dma_start(w8lo_t[:], w8lo)
            w8xp_t = wp.tile([64, 3 * C], F8, name="w8xp_t")
            nc.sync.dma_start(w8xp_t[:], w8xp)
            ddg8_t = wp.tile([128, 9 * 128], F8, name="ddg8_t")
            nc.sync.dma_start(ddg8_t[:], ddg8)
            prA = load_cast(projT[0:96, :], 96, C, "prA")
            prB = load_cast(projT[96:192, :], 96, C, "prB")
            dws_t = wp.tile([128, 9 * 3], F32)  # 3 chunks side by side: [:,9m+t]
            for m in range(3):
                nc.sync.dma_start(dws_t[0:128, 9 * m:9 * m + 9], dws[128 * m:128 * m + 128, :])
            tauA = wp.tile([128, 1], F32)
            nc.sync.dma_start(tauA[:], tau[0:128, :])
            tauB = wp.tile([64, 1], F32)
            nc.sync.dma_start(tauB[:], tau[128:192, :])
            identF = wp.tile([128, 128], F32)
            make_identity(nc, identF[:])
            stg_ctx.close()

            # ssq accumulator slots (per stripe), fp32
            ssqA = wp.tile([128, NS], F32)   # k ch 0..127
            ssqB = wp.tile([64, NS], F32)    # k ch 128..191

            # persistent v ch 64..191 in SBUF (bf16); ch 0..63 spills to vscrM
            vsbB = wp.tile([128, HW], BF16, name="vsbB")

            # persistent psum: attn raw blocks + q gram
            raw01 = pp.tile([96, 96], F32)
            raw23 = pp.tile([96, 96], F32)
            gq_hi = pp.tile([128, 128], F32)
            gq_lo = pp.tile([64, 64], F32)

            with ExitStack() as sctx:
                sp = sctx.enter_context(tc.tile_pool(name="sp", bufs=2))      # stripe transients
                sp1 = sctx.enter_context(tc.tile_pool(name="sp1", bufs=1))    # single-buffered stages
                qp = sctx.enter_context(tc.tile_pool(name="qp", bufs=4))
                kvpool = sctx.enter_context(tc.tile_pool(name="kvp", bufs=2))
                kvpool3 = sctx.enter_context(tc.tile_pool(name="kvp3", bufs=3))
                pk = sctx.enter_context(tc.tile_pool(name="pk", bufs=2, space="PSUM"))
                pq = sctx.enter_context(tc.tile_pool(name="pq", bufs=2, space="PSUM"))

                # per-stripe produce: stage inputs, kv conv, casts/packs, dw, kt
                def produce(s):
                    r_lo = SR * s - 1           # first (halo) image row
                    n_lo = r_lo * W

                    # ---- x DMA (f32r, fed to PE directly) ----
                    xA32 = sp.tile([128, PR * W], XDT, tag="xA32", name="xA32")
                    xB32 = sp.tile([64, PR * W], XDT, tag="xB32", name="xB32")
                    for t, p0, p in ((xA32, 0, 128), (xB32, 128, 64)):
                        if s == 0:
                            nc.sync.dma_start(t[:, 0:W], zpad[0:p, :])
                            nc.sync.dma_start(t[:, W:], x[p0:p0 + p, 0:(PR - 1) * W])
                        elif s == NS - 1:
                            nc.sync.dma_start(t[:, (PR - 1) * W:], zpad[0:p, :])
                            nc.sync.dma_start(t[:, 0:(PR - 1) * W], x[p0:p0 + p, n_lo:n_lo + (PR - 1) * W])
                        else:
                            nc.sync.dma_start(t[:], x[p0:p0 + p, n_lo:n_lo + PR * W])

                    if not kv_f32r:
                        xbA = sp.tile([128, PR * W], BF16, tag="xbA")
                        nc.gpsimd.tensor_copy(xbA[:], xA32[:])
                        xbB = sp.tile([64, PR * W], BF16, tag="xbB")
                        nc.gpsimd.tensor_copy(xbB[:], xB32[:])
                        kv_in_A, kv_in_B = xbA, xbB
                    else:
                        kv_in_A, kv_in_B = xA32, xB32

                    # ---- y DMA (f32) ----
                    yA32 = sp1.tile([128, PR * W], F32, tag="yA32", name="yA32")
                    if s == 0:
                        nc.gpsimd.memset(yA32[:, 0:W], 0.0)
                        nc.sync.dma_start(yA32[:, W:], y[0:128, 0:(PR - 1) * W])
                    elif s == NS - 1:
                        nc.gpsimd.memset(yA32[:, (PR - 1) * W:], 0.0)
                        nc.sync.dma_start(yA32[:, 0:(PR - 1) * W], y[0:128, n_lo:n_lo + (PR - 1) * W])
                    else:
                        nc.sync.dma_start(yA32[:], y[0:128, n_lo:n_lo + PR * W])
                    # rows 16s-2 .. 16s+16 (19 rows) for the row-shifted packs
                    yB19 = sp1.tile([64, (PR + 1) * W], F32, tag="yB19", name="yB19")
                    lo2 = (SR * s - 2) * W
                    a, b = 0, (PR + 1) * W
                    if s == 0:
                        nc.gpsimd.memset(yB19[:, 0:2 * W], 0.0)
                        a = 2 * W
                    if s == NS - 1:
                        nc.gpsimd.memset(yB19[:, (PR + 1 - 1) * W:], 0.0)
                        b = (PR + 1 - 1) * W
                    nc.sync.dma_start(yB19[:, a:b], y[128:192, lo2 + a:lo2 + b])

                    # ---- y casts (DVE) ----
                    ypA = sp.tile([128, PR, PWQ], F8, tag="ypA", name="ypA")
                    nc.gpsimd.memset(ypA[:, :, 0:1], 0.0)
                    nc.gpsimd.memset(ypA[:, :, 1 + W:3 + W], 0.0)
                    nc.scalar.copy(
                        ypA[:, :, 1:1 + W],
                        yA32[:].rearrange("p (a b) -> p a b", b=W))
                    ybB = sp1.tile([64, PR + 1, W], F8, tag="ybB", name="ybB")
                    nc.scalar.copy(ybB[:], yB19[:].rearrange("p (a b) -> p a b", b=W))

                    # ---- packed y-lo tile via local DMA dups ----
                    # ypB: lower = ybB @col1 (taps (dy,0) and, read at col+2, (dy,+1)),
                    #      upper = ybB @col2 (taps (dy,-1))
                    ypB = sp.tile([128, PR + 1, PWQ], F8, tag="ypB", name="ypB")
                    nc.gpsimd.memset(ypB[0:64, :, 0:1], 0.0)
                    nc.gpsimd.memset(ypB[0:64, :, 1 + W:3 + W], 0.0)
                    nc.gpsimd.memset(ypB[64:128, :, 0:2], 0.0)
                    nc.gpsimd.memset(ypB[64:128, :, 2 + W:3 + W], 0.0)
                    nc.gpsimd.dma_start(ypB[0:64, :, 1:1 + W], ybB[:])
                    nc.gpsimd.dma_start(ypB[64:128, :, 2:2 + W], ybB[:])

                    # ---- kv 1x1 conv (fp32r) -> padded kvp chunks ----
                    kvp = []
                    for m in range(3):
                        if m == 0:
                            kvt = kvpool.tile([128, PR, PWQ], F8, tag=f"kvp{m}")
                            nc.gpsimd.memset(kvt[:, :, 0:1], 0.0)
                            nc.gpsimd.memset(kvt[:, :, 1 + W:3 + W], 0.0)
                        else:
                            kvt = kvpool3.tile([128, PR, PW], BF16, tag=f"kvp{m}")
                            nc.gpsimd.memset(kvt[:, :, 0:1], 0.0)
                            nc.gpsimd.memset(kvt[:, :, PW - 1:PW], 0.0)
                        kvp.append(kvt)
                        lhs_hi = wkv_hi[:, 128 * m:128 * m + 128]
                        lhs_lo = wkv_lo[:, 128 * m:128 * m + 128]
                        for j in range(0, PR * W, 512):
                            w_ = min(512, PR * W - j)
                            pst = pk.tile([128, 512], F32, tag="pkv", name="pkv")
                            ps = pst[:, 0:w_]
                            nc.tensor.matmul(ps, lhs_hi, kv_in_A[:, j:j + w_], start=True, stop=False)
                            nc.tensor.matmul(ps, lhs_lo, kv_in_B[:, j:j + w_], start=False, stop=True)
                            if m == 0:
                                nc.scalar.activation(
                                    kvt[:, j // W:j // W + w_ // W, 1:1 + W],
                                    ps.rearrange("p (a b) -> p a b", b=W),
                                    AF.Copy, scale=8.0)
                            else:
                                nc.scalar.copy(
                                    kvt[:, j // W:j // W + w_ // W, 1:1 + W],
                                    ps.rearrange("p (a b) -> p a b", b=W))

                    # ---- depthwise 3x3: m0 on PE (fp8 dwdiag, DoubleRow dy-pairs),
                    #      m1/m2 elementwise on DVE/Pool ----
                    kA = sp.tile([128, SR, W], BF16, tag="kA")
                    for jr in range(0, SR, 4):
                        pdw = pk.tile([128, 512], F32, tag="pkv", name="pdw")
                        for d, dx in enumerate((-1, 0, 1)):
                            # rhs [K, j=2(dy -1/0, step PWQ), row=4, col=W]
                            b4 = kvp[0][:, jr:jr + 4, 1 + dx:1 + dx + W].unsqueeze(1)
                            b4 = b4.broadcast_to([128, 2, 4, W])
                            v4 = b4.ap
                            v4[1] = [PWQ, 2]
                            b4.ap = v4
                            lhsp = ddg8_t[:, d * 256:(d + 1) * 256].rearrange(
                                "p (j m) -> p j m", j=2)
                            nc.tensor.matmul(pdw[:], lhsp, b4, start=(d == 0), stop=False,
                                             perf_mode=PM.DoubleRow)
                        for d, dx in enumerate((-1, 0, 1)):
                            rhs1 = kvp[0][:, jr + 2:jr + 6, 1 + dx:1 + dx + W]
                            lhs1 = ddg8_t[:, 768 + d * 128:768 + (d + 1) * 128]
                            nc.tensor.matmul(pdw[:], lhs1, rhs1, start=False, stop=(d == 2))
                        nc.scalar.copy(kA[:, jr:jr + 4, :],
                                       pdw[:].rearrange("p (a b) -> p a b", b=W))
                    kvmid = sp.tile([128, SR, W], BF16, tag="kvmid")
                    vBv = vsbB[:, SR * s * W:(SR * s + SR) * W].rearrange("p (a b) -> p a b", b=W)
                    douts = [None, kvmid[:, :, :], vBv]
                    for m in range(1, 3):
                        dst = douts[m]
                        tmp = sp.tile([128, SR, W], BF16, tag="dwtmp", name="dwtmp")
                        for ti, (dy, dx) in enumerate(taps):
                            sc = dws_t[:, 9 * m + ti:9 * m + ti + 1]
                            src = kvp[m][:, 1 + dy:1 + SR + dy, 1 + dx:1 + dx + W]
                            for eng, c0, c1 in ((nc.vector, 0, DWS), (nc.gpsimd, DWS, W)):
                                if c0 >= c1:
                                    continue
                                d = dst[:, :, c0:c1]
                                sl = src[:, :, c0:c1]
                                if ti == 0:
                                    eng.tensor_scalar_mul(d, sl, sc)
                                else:
                                    t_ = tmp[:, :, c0:c1]
                                    eng.tensor_scalar_mul(t_, sl, sc)
                                    eng.tensor_add(d, d, t_)

                    # ---- k transpose: kt[p, r, c] = k[c, r*W + p] ----
                    kt = sp.tile([128, SR, C], BF16, tag="kt")
                    nc.scalar.dma_start_transpose(kt[:, :, 0:128], kA[:].rearrange("p a b -> p (a b)"))
                    nc.scalar.dma_start_transpose(kt[:, :, 128:192], kvmid[0:64].rearrange("p a b -> p (a b)"))

                    # ---- v ch 0..63 spill to DRAM ----
                    nsl = slice(SR * s * W, SR * s * W + SR * W)
                    nc.scalar.dma_start(vscrM[:, nsl], kvmid[64:128].rearrange("p a b -> p (a b)"))

                    return ypA, ypB, kt, kA, kvmid

                def produce_tail(s, st):
                    # ssq_k (in-place square; emitted after consume so the Act
                    # queue isn't blocked between kvp and qsb copies)
                    _, _, _, kA, kvmid = st
                    nc.scalar.activation(kA[:], kA[:], AF.Square, accum_out=ssqA[:, s:s + 1])
                    nc.scalar.activation(kvmid[0:64], kvmid[0:64], AF.Square, accum_out=ssqB[:, s:s + 1])

                # ---- dense conv (fused q) + attn/gram accumulation for stripe s ----
                def consume(s, st):
                    ypA, ypB, kt = st[0], st[1], st[2]

                    def attn_mms(qsb, r, kt_):
                        f = (s == 0 and r == 0)
                        l = (s == NS - 1 and r == SR - 1)
                        nc.tensor.matmul(gq_hi[:], qsb[:, 0:128], qsb[:, 0:128], start=f, stop=l)
                        nc.tensor.matmul(gq_lo[:], qsb[:, 128:192], qsb[:, 128:192], start=f, stop=l)
                        nc.tensor.matmul(raw01[:], qsb[:, 0:96], kt_[:, r, 0:96], start=f, stop=l)
                        nc.tensor.matmul(raw23[:], qsb[:, 96:192], kt_[:, r, 96:192], start=f, stop=l)

                    pends = []
                    for r in range(SR):
                        psq = pq.tile([128, C], F32, tag="psq")
                        # y-hi: 3 dy-pair DoubleRows + 3 dy=+1 singles
                        for d, dx in enumerate((-1, 0, 1)):
                            lhs = ypA[:, r:r + 2, 1 + dx:1 + dx + W]
                            rhs = w8hi_t[:, d * 2 * C:(d * 2 + 2) * C].rearrange(
                                "p (j n) -> p j n", j=2)
                            nc.tensor.matmul(psq, lhs, rhs, start=(d == 0), stop=False,
                                             perf_mode=PM.DoubleRow)
                        for d, dx in enumerate((-1, 0, 1)):
                            nc.tensor.matmul(psq, ypA[:, r + 2, 1 + dx:1 + dx + W],
                                             w8hi_t[:, (6 + d) * C:(7 + d) * C],
                                             start=False, stop=False)
                        # y-lo packed {lower=(dy,0), upper=(dy,-1)}: dy-pair DR + single
                        rhs = w8lo_t[:, 0:2 * C].rearrange("p (j n) -> p j n", j=2)
                        nc.tensor.matmul(psq, ypB[:, r + 1:r + 3, 1:1 + W], rhs,
                                         start=False, stop=False, perf_mode=PM.DoubleRow)
                        nc.tensor.matmul(psq, ypB[:, r + 3, 1:1 + W], w8lo_t[:, 2 * C:3 * C],
                                         start=False, stop=False)
                        # y-lo dx=+1 (reads @col3): dy-pair DR (K=64) + single
                        rhs = w8xp_t[:, 0:2 * C].rearrange("p (j n) -> p j n", j=2)
                        nc.tensor.matmul(psq, ypB[0:64, r + 1:r + 3, 3:3 + W], rhs,
                                         start=False, stop=False, perf_mode=PM.DoubleRow)
                        nc.tensor.matmul(psq, ypB[0:64, r + 3, 3:3 + W], w8xp_t[:, 2 * C:3 * C],
                                         start=False, stop=True)
                        qsb = qp.tile([128, C], BF16, tag="qsb")
                        nc.scalar.copy(qsb[:], psq[:])
                        pends.append((qsb, r, kt))
                        if len(pends) > 2:
                            attn_mms(*pends.pop(0))
                        if r == SR - 1:
                            for p_ in pends:
                                attn_mms(*p_)
                            pends = []

                # software pipeline: produce stripe i, consume stripe i-1
                prev = None
                for i in range(NS + 1):
                    cur = produce(i) if i < NS else None
                    if prev is not None:
                        consume(i - 1, prev)
                        produce_tail(i - 1, prev)
                    prev = cur

            # ================= phase 2: softmax + MT =================
            mt_M = wp.tile([64, C], BF16, name="mt_M")    # rows = v ch 0..63
            mt_B = wp.tile([128, C], BF16, name="mt_B")   # rows = v ch 64..191
            with ExitStack() as sctx2:
                s2 = sctx2.enter_context(tc.tile_pool(name="s2", bufs=1))
                p2 = sctx2.enter_context(tc.tile_pool(name="p2", bufs=1, space="PSUM"))

                # ssq_q from gram diagonals
                scr2 = s2.tile([128, 128], F32)
                ssqqA = s2.tile([128, 1], F32)
                nc.vector.scalar_tensor_tensor(scr2[:], gq_hi[:], 1.0, identF[:],
                                               AL.mult, AL.mult, accum_out=ssqqA[:])
                scr2b = s2.tile([64, 64], F32)
                ssqqB = s2.tile([64, 1], F32)
                nc.vector.scalar_tensor_tensor(scr2b[:], gq_lo[:], 1.0, identF[0:64, 0:64],
                                               AL.mult, AL.mult, accum_out=ssqqB[:])
                # ssq_k totals
                sskA = s2.tile([128, 1], F32)
                nc.vector.reduce_sum(sskA[:], ssqA[:], axis=mybir.AxisListType.X)
                sskB = s2.tile([64, 1], F32)
                nc.vector.reduce_sum(sskB[:], ssqB[:], axis=mybir.AxisListType.X)

                def rsqrt(dst, src):
                    nc.vector.reciprocal(dst, src)
                    nc.scalar.activation(dst, dst, AF.Sqrt)

                rqA = s2.tile([128, 1], F32, name="rqA")
                rsqrt(rqA[:], ssqqA[:])
                rqB = s2.tile([64, 1], F32, name="rqB")
                rsqrt(rqB[:], ssqqB[:])
                rkA = s2.tile([128, 1], F32, name="rkA")
                rsqrt(rkA[:], sskA[:])
                rkB = s2.tile([64, 1], F32, name="rkB")
                rsqrt(rkB[:], sskB[:])
                # rq * tau
                nc.vector.tensor_mul(rqA[:], rqA[:], tauA[:])
                nc.vector.tensor_mul(rqB[:], rqB[:], tauB[:])

                # rk rows [1, 192] via DRAM bounce (SBUF partition-transpose DMA is illegal)
                nc.scalar.dma_start(rscr[0:1, 0:128].rearrange("a b -> b a"), rkA[:])
                nc.scalar.dma_start(rscr[0:1, 128:192].rearrange("a b -> b a"), rkB[:])
                rkrow = s2.tile([1, 192], F32)
                nc.scalar.dma_start(rkrow[:], rscr)
                rkrow_b = s2.tile([1, 192], BF16)
                nc.vector.tensor_copy(rkrow_b[:], rkrow[:])
                ones1 = s2.tile([1, 96], BF16)
                nc.vector.memset(ones1[:], 1.0)
                rkb01p = p2.tile([96, 96], F32)
                nc.tensor.matmul(rkb01p[:], ones1[:], rkrow_b[0:1, 0:96], start=True, stop=True)
                rkb23p = p2.tile([96, 96], F32)
                nc.tensor.matmul(rkb23p[:], ones1[:], rkrow_b[0:1, 96:192], start=True, stop=True)

                # logits = raw * (rq*tau) * rk
                l01 = s2.tile([96, 96], F32)
                nc.scalar.activation(l01[:], raw01[:], AF.Copy, scale=rqA[0:96, :])
                nc.vector.tensor_mul(l01[:], l01[:], rkb01p[:])
                l23 = s2.tile([96, 96], F32)
                rq23 = s2.tile([96, 1], F32)
                nc.scalar.dma_start(rq23[0:32, :], rqA[96:128, :])
                nc.scalar.dma_start(rq23[32:96, :], rqB[:])
                nc.scalar.activation(l23[:], raw23[:], AF.Copy, scale=rq23[:])
                nc.vector.tensor_mul(l23[:], l23[:], rkb23p[:])

                # softmax per head-pair with additive block mask -> blockdiag bd (bf16)
                msk = s2.tile([96, 96], F32)
                nc.sync.dma_start(msk[:], bmask)
                bd01 = s2.tile([96, 96], BF16)
                bd23 = s2.tile([96, 96], BF16)
                for hb, (lt, bd) in enumerate(((l01, bd01), (l23, bd23))):
                    nc.vector.tensor_add(lt[:], lt[:], msk[:])
                    mx = s2.tile([96, 1], F32, tag=f"mx{hb}", name=f"mx{hb}")
                    nc.vector.reduce_max(mx[:], lt[:], axis=mybir.AxisListType.X)
                    nc.vector.tensor_scalar_mul(mx[:], mx[:], -1.0)
                    ex = s2.tile([96, 96], F32, tag=f"ex{hb}", name=f"ex{hb}")
                    rs = s2.tile([96, 1], F32, tag=f"rs{hb}", name=f"rs{hb}")
                    nc.scalar.activation(ex[:], lt[:], AF.Exp, bias=mx[:], accum_out=rs[:])
                    nc.vector.reciprocal(rs[:], rs[:])
                    nc.vector.tensor_scalar_mul(bd[:], ex[:], rs[:])

                # MT[d, o] = sum_c attn[c, d] * projT[c, o], laid out to match v tiles
                mtM_p = p2.tile([64, C], F32)
                nc.tensor.matmul(mtM_p[:], bd01[:, 0:64], prA[:], start=True, stop=True)
                mtB_p = p2.tile([128, C], F32)
                nc.tensor.matmul(mtB_p[0:32, :], bd01[:, 64:96], prA[:], start=True, stop=True)
                nc.tensor.matmul(mtB_p[32:64, :], bd23[:, 0:32], prB[:], start=True, stop=True)
                nc.tensor.matmul(mtB_p[64:128, :], bd23[:, 32:96], prB[:], start=True, stop=True)
                nc.scalar.copy(mt_M[:], mtM_p[:])
                nc.scalar.copy(mt_B[:], mtB_p[:])
            pp_ctx.close()

            # ================= phase 3: out = MT.T @ v from SBUF =================
            with ExitStack() as sctx3:
                s3 = sctx3.enter_context(tc.tile_pool(name="s3", bufs=3))
                p3 = sctx3.enter_context(tc.tile_pool(name="p3", bufs=3, space="PSUM"))
                for j in range(NCK):
                    nsl = slice(512 * j, 512 * j + 512)
                    vM = s3.tile([64, 512], BF16, tag="vM")
                    nc.sync.dma_start(vM[:], vscrM[:, nsl])
                    f1 = p3.tile([128, 512], F32, tag="f1")
                    nc.tensor.matmul(f1[:], mt_M[:, 0:128], vM[:], start=True, stop=False)
                    nc.tensor.matmul(f1[:], mt_B[:, 0:128], vsbB[:, nsl], start=False, stop=True)
                    f2 = p3.tile([64, 512], F32, tag="f2")
                    nc.tensor.matmul(f2[:], mt_M[:, 128:192], vM[:], start=True, stop=False)
                    nc.tensor.matmul(f2[:], mt_B[:, 128:192], vsbB[:, nsl], start=False, stop=True)
                    o1 = s3.tile([128, 512], F32, tag="o1")
                    nc.vector.tensor_copy(o1[:], f1[:])
                    o2 = s3.tile([64, 512], F32, tag="o2")
                    nc.scalar.copy(o2[:], f2[:])
                    nc.sync.dma_start(out[0:128, nsl], o1[:])
                    nc.sync.dma_start(out[128:192, nsl], o2[:])

    nc.compile()
    return nc


# ======================= harness entry point =======================
B = 8
H = 128
_NC = None


def _get_nc():
    global _NC
    if _NC is None:
        _NC = build(H=H)
    return _NC


def _make_in_maps(inputs):
    x = np.ascontiguousarray(inputs["x"], np.float32)
    y = np.ascontiguousarray(inputs["y"], np.float32)
    prep = host_prep(inputs["kv_w"], inputs["kv_dw_w"], inputs["q_w"],
                     inputs["q_dw_w"], inputs["proj_w"], inputs["temperature"])
    maps = []
    for b in range(B):
        m = {"x": x[b].reshape(C, H * W), "y": y[b].reshape(C, H * W)}
        m.update(prep)
        maps.append(m)
    return maps


def _run(inputs, trace=False, trace_kwargs=None):
    from concourse.bass_utils import run_bass_kernel_spmd
    nc = _get_nc()
    res = run_bass_kernel_spmd(nc, _make_in_maps(inputs), core_ids=list(range(B)),
                               trace=trace, trace_kwargs=trace_kwargs or {})
    out = np.stack([np.asarray(res.results[b]["out"], np.float32).reshape(C, H, W)
                    for b in range(B)])
    return out, res


def kernel(**inputs) -> np.ndarray:
    out, _ = _run(inputs, trace=False)
    return out
